# revision 11
# baseline (speedup 1.0000x reference)
"""HypergraphConv v8: v6 + stage-A pool scoped out to deepen G buffering.

The xT tiles free their 24KB/partition before the gather pool opens, raising
the outstanding-gather window from 20 to 24 buffers.

Phase D also runs per edge-half: pass 0 gathers only from the first half of
the m-table (deps on AG0 alone, so it overlaps AG1 and the phase-B tail),
parking per-tile partials in an SBUF bf16 accumulator; pass 1 adds them back
before the fused scale+relu.

Phase B runs in two edge-space halves; each half's ReduceScatter/rescale/
AllGather issues as soon as its partials are written, so the first half's
collective latency overlaps the second half's gather/compute work.

All stage pools open once per rep (shared G pool for both scatter phases,
fixed-size G tiles), removing the per-stage pool drain barriers that
serialized the v2 timeline.
"""

import numpy as np
import ml_dtypes
from contextlib import ExitStack

import concourse.bacc as bacc
import concourse.bass as bass
import concourse.mybir as mybir
import concourse.tile as tile
from concourse import library_config
from concourse.bass_utils import run_bass_kernel_spmd

NCORES = 8
P = 128

N_NODES = 50000
N_EDGES = 20000
IN_DIM = 256
OUT_DIM = 128

W1 = 64
W2 = 64
GROUP = 16

BF16 = mybir.dt.bfloat16
F32 = mybir.dt.float32
I16 = mybir.dt.int16

PAD_OH = 200.0


def _derived():
    npc = N_NODES // NCORES
    e_pad = -(-N_EDGES // (NCORES * P)) * (NCORES * P)
    n_t1 = e_pad // W1
    eslice = e_pad // NCORES
    n_t2 = -(-npc // W2)
    return npc, e_pad, n_t1, eslice, n_t2


def _wrap_idx16(idx):
    a = np.asarray(idx, dtype=np.int16).reshape(-1, 16).T
    return np.ascontiguousarray(np.tile(a, (8, 1)))


def _oh_cols(oh):
    return np.ascontiguousarray(oh.reshape(-1, P).T.astype(ml_dtypes.bfloat16))


def _bucket_entries(gidx, oh, tid, n_tiles, chunks):
    order = np.argsort(tid, kind="stable")
    gidx = gidx[order]
    oh = oh[order]
    tid_s = tid[order]
    counts = np.bincount(tid_s, minlength=n_tiles)
    starts = np.concatenate([[0], np.cumsum(counts[:-1])])
    dest_base = np.concatenate([[0], np.cumsum(chunks[:-1])]) * P
    L = int(chunks.sum()) * P
    g_out = np.zeros(L, dtype=np.int64)
    oh_out = np.full(L, PAD_OH, dtype=np.float32)
    n = gidx.shape[0]
    rank = np.arange(n, dtype=np.int64) - starts[tid_s]
    dest = dest_base[tid_s] + rank
    g_out[dest] = gidx
    oh_out[dest] = oh
    return g_out, oh_out, L


def _make_groups(chunks):
    groups = []
    t = 0
    n_tiles = len(chunks)
    cbase = 0
    while t < n_tiles:
        nch = 0
        t0 = t
        while t < n_tiles and (nch == 0 or nch + chunks[t] <= GROUP):
            nch += int(chunks[t])
            t += 1
        groups.append((t0, t, cbase, nch))
        cbase += nch
    return groups


def build_kernel(chunks1, chunks2, bias_nz, reps=1):
    import os
    n_queues = int(os.environ.get("V2_QUEUES", "4"))
    gbufs = int(os.environ.get("V2_GBUFS", "24"))
    global GROUP
    GROUP = int(os.environ.get("V2_GROUP", "16"))
    npc, e_pad, n_t1, eslice, n_t2 = _derived()
    LA = int(np.sum(chunks1)) * P
    LB = int(np.sum(chunks2)) * P
    NCA = LA // P
    NCB = LB // P
    n_t1_half = n_t1 // 2
    groups1_h = []
    cb = 0
    for h in range(2):
        ch = chunks1[h * n_t1_half:(h + 1) * n_t1_half]
        gs = [(t_lo + h * n_t1_half, t_hi + h * n_t1_half, gc0 + cb, nch)
              for (t_lo, t_hi, gc0, nch) in _make_groups(ch)]
        groups1_h.append(gs)
        cb += int(np.sum(ch))
    n_t2_d = len(chunks2) // 2   # chunks2 holds both halves
    groups2_h = []
    cb2 = 0
    for h in range(2):
        ch = chunks2[h * n_t2_d:(h + 1) * n_t2_d]
        gs = [(t_lo, t_hi, gc0 + cb2, nch)
              for (t_lo, t_hi, gc0, nch) in _make_groups(ch)]
        groups2_h.append(gs)
        cb2 += int(np.sum(ch))

    nc = bacc.Bacc("TRN2", num_devices=NCORES, num_swdge_queues=n_queues)

    xT_in = nc.dram_tensor("xT", [IN_DIM, npc], BF16, kind="ExternalInput")
    w_in = nc.dram_tensor("w", [IN_DIM, OUT_DIM], BF16, kind="ExternalInput")
    bias_in = nc.dram_tensor("bias", [1, OUT_DIM], F32, kind="ExternalInput")
    idxA_in = nc.dram_tensor("idxA", [P, LA // 16], I16, kind="ExternalInput")
    ohA_in = nc.dram_tensor("ohA", [P, NCA], BF16, kind="ExternalInput")
    idxB_in = nc.dram_tensor("idxB", [P, LB // 16], I16, kind="ExternalInput")
    ohB_in = nc.dram_tensor("ohB", [P, NCB], BF16, kind="ExternalInput")
    binv_in = nc.dram_tensor("binv", [P, eslice // P], F32, kind="ExternalInput")
    dinv_in = nc.dram_tensor("dinv", [P, n_t2], F32, kind="ExternalInput")
    out_part = nc.dram_tensor("out_part", [OUT_DIM, 1], F32, kind="ExternalOutput")

    xw_tab = nc.dram_tensor("xw_tab", [npc, OUT_DIM], BF16)
    m_part = nc.dram_tensor("m_part", [e_pad, OUT_DIM], F32)
    half_rows = e_pad // 2
    esl_h = half_rows // NCORES
    n_t1_h = n_t1 // 2
    m_red_h = [nc.dram_tensor(f"m_red{h}", [esl_h, OUT_DIM], F32)
               for h in range(2)]
    mtab_s_h = [nc.dram_tensor(f"mtab_s{h}", [esl_h, OUT_DIM], BF16)
                for h in range(2)]
    mtab = nc.dram_tensor("mtab", [e_pad, OUT_DIM], BF16, addr_space="Shared")

    with tile.TileContext(nc) as tc, ExitStack() as ctx:
        pin = ctx.enter_context(tc.tile_pool(name="pin", bufs=1))

        nc.gpsimd.load_library(library_config.mlp)

        iota_i = pin.tile([P, P], I16)
        iota_bf = pin.tile([P, P], BF16)
        nc.gpsimd.iota(iota_i[:], [[1, P]], channel_multiplier=0)
        nc.vector.tensor_copy(out=iota_bf[:], in_=iota_i[:])
        ones_f32 = pin.tile([P, 1], F32)
        nc.vector.memset(ones_f32[:], 1.0)
        binv_sb = pin.tile([P, eslice // P], F32)
        dinv_sb = pin.tile([P, n_t2], F32)
        nc.sync.dma_start(out=binv_sb[:], in_=binv_in[:])
        nc.sync.dma_start(out=dinv_sb[:], in_=dinv_in[:])
        idxA = pin.tile([P, LA // 16], I16)
        ohA = pin.tile([P, NCA], BF16)
        idxB = pin.tile([P, LB // 16], I16)
        ohB = pin.tile([P, NCB], BF16)
        nc.sync.dma_start(out=idxA[:], in_=idxA_in[:])
        nc.sync.dma_start(out=ohA[:], in_=ohA_in[:])
        nc.sync.dma_start(out=idxB[:], in_=idxB_in[:])
        nc.sync.dma_start(out=ohB[:], in_=ohB_in[:])
        if bias_nz:
            bias_bc = pin.tile([P, OUT_DIM], F32)
            nc.sync.dma_start(
                out=bias_bc[:], in_=bass.AP(bias_in, 0, [[0, P], [1, OUT_DIM]]))

        def s_build(S_tile, oh_tile, col0, k, w):
            s_ap = S_tile[:, :k * w].rearrange("p (k j) -> p k j", k=k)
            o = oh_tile[:, col0:col0 + k]
            in0 = bass.AP(o.tensor, o.offset, [list(o.ap[0]), list(o.ap[1]), [0, w]])
            it = iota_bf[:, :w]
            in1 = bass.AP(it.tensor, it.offset, [list(it.ap[0]), [0, k], [1, w]])
            nc.vector.tensor_tensor(out=s_ap, in0=in0, in1=in1,
                                    op=mybir.AluOpType.is_equal)

        qrr = [0]
        SMAX = max(max(int(c) for c in chunks1), max(int(c) for c in chunks2))
        GMAX = max(GROUP, SMAX)

        def one_rep():
            with tc.tile_pool(name="pacc", bufs=1) as pacc, \
                 tc.tile_pool(name="pw", bufs=3) as pw, \
                 tc.tile_pool(name="psa", bufs=2, space="PSUM") as psa, \
                 tc.tile_pool(name="psb", bufs=2, space="PSUM") as psb, \
                 tc.tile_pool(name="psc", bufs=1, space="PSUM") as psc:

                # ---- stage A: xw table = x @ W (bf16); pool scoped -----
                with tc.tile_pool(name="pa", bufs=1) as pa:
                    kh = IN_DIM // P
                    xT_sb = [pa.tile([P, npc], BF16, tag=f"xT{k}", name=f"xT{k}")
                             for k in range(kh)]
                    w_sb = [pa.tile([P, OUT_DIM], BF16, tag=f"w{k}", name=f"w{k}")
                            for k in range(kh)]
                    for k in range(kh):
                        nc.sync.dma_start(out=xT_sb[k][:],
                                          in_=xT_in[k * P:(k + 1) * P, :])
                        nc.sync.dma_start(out=w_sb[k][:],
                                          in_=w_in[k * P:(k + 1) * P, :])
                    for i in range(0, npc, P):
                        nt = min(P, npc - i)
                        pxw = psa.tile([P, OUT_DIM], F32, tag="pxw", name="pxw")
                        for k in range(kh):
                            nc.tensor.matmul(
                                out=pxw[:nt], lhsT=xT_sb[k][:, i:i + nt],
                                rhs=w_sb[k][:],
                                start=(k == 0), stop=(k == kh - 1))
                        st = pw.tile([P, OUT_DIM], BF16, tag="xst", name="xst")
                        nc.scalar.copy(out=st[:nt], in_=pxw[:nt])
                        nc.sync.dma_start(out=xw_tab[i:i + nt, :], in_=st[:nt, :])

                ctx2 = ExitStack()
                pg = ctx2.enter_context(tc.tile_pool(name="pg", bufs=gbufs))

                def gather_group(tab_ap, idx_sb, c0, nch, tag):
                    G = pg.tile([P, GMAX, OUT_DIM], BF16, tag=tag, name=tag)
                    for g0 in range(0, nch, GROUP):
                        gk = min(GROUP, nch - g0)
                        nc.gpsimd.dma_gather(
                            G[:, g0:g0 + gk, :], tab_ap,
                            idx_sb[:, (c0 + g0) * 8:(c0 + g0 + gk) * 8],
                            gk * P, gk * P, OUT_DIM, single_packet=False,
                            queue_num=qrr[0])
                        qrr[0] = (qrr[0] + 1) % n_queues
                    return G

                # ---- stage B: per-half scatter + RS/scale/AG -----------
                for h in range(2):
                    for (t_lo, t_hi, gc0, nch) in groups1_h[h]:
                        G = gather_group(xw_tab[:, :], idxA, gc0, nch, "G")
                        cbase = gc0
                        for t in range(t_lo, t_hi):
                            kt = int(chunks1[t])
                            S = pw.tile([P, SMAX * W1], BF16, tag="S", name="S")
                            s_build(S, ohA, cbase, kt, W1)
                            pm = psb.tile([P, OUT_DIM], F32, tag="pm", name="pm")
                            for c in range(kt):
                                nc.tensor.matmul(
                                    out=pm[:W1],
                                    lhsT=S[:, c * W1:(c + 1) * W1],
                                    rhs=G[:, cbase - gc0 + c, :],
                                    start=(c == 0), stop=(c == kt - 1),
                                    skip_group_check=True)
                            mt = pw.tile([P, OUT_DIM], F32, tag="mt", name="mt")
                            nc.scalar.copy(out=mt[:W1], in_=pm[:W1])
                            nc.sync.dma_start(
                                out=m_part[t * W1:(t + 1) * W1, :], in_=mt[:W1, :])
                            cbase += kt
                    nc.gpsimd.collective_compute(
                        "ReduceScatter", mybir.AluOpType.add,
                        replica_groups=[list(range(NCORES))],
                        ins=[m_part[h * half_rows:(h + 1) * half_rows, :]],
                        outs=[m_red_h[h][:, :]])
                    for ts in range(esl_h // P):
                        mc = pw.tile([P, OUT_DIM], F32, tag="mc", name="mc")
                        nc.sync.dma_start(
                            out=mc[:], in_=m_red_h[h][ts * P:(ts + 1) * P, :])
                        ms = pw.tile([P, OUT_DIM], BF16, tag="ms", name="ms")
                        nc.scalar.activation(
                            out=ms[:], in_=mc[:],
                            func=mybir.ActivationFunctionType.Copy,
                            scale=binv_sb[:, h * (esl_h // P) + ts:
                                          h * (esl_h // P) + ts + 1])
                        nc.sync.dma_start(
                            out=mtab_s_h[h][ts * P:(ts + 1) * P, :], in_=ms[:])
                    nc.gpsimd.collective_compute(
                        "AllGather", mybir.AluOpType.bypass,
                        replica_groups=[list(range(NCORES))],
                        ins=[mtab_s_h[h][:, :]],
                        outs=[mtab[h * half_rows:(h + 1) * half_rows, :]])

                # ---- stage D: two passes over edge halves --------------
                acc = pacc.tile([P, n_t2 * OUT_DIM], BF16, tag="acc", name="acc")
                pcol = psc.tile([P, 1], F32, name="pcol")
                first = True
                for h in range(2):
                    tab_ap = mtab[h * half_rows:(h + 1) * half_rows, :]
                    for (t_lo, t_hi, gc0, nch) in groups2_h[h]:
                        G = gather_group(tab_ap, idxB, gc0, nch, "G")
                        cbase = gc0
                        for tt in range(t_lo, t_hi):
                            kt = int(chunks2[h * n_t2_d + tt])
                            nt = min(W2, npc - tt * W2)
                            S = pw.tile([P, SMAX * W2], BF16, tag="S", name="S")
                            s_build(S, ohB, cbase, kt, W2)
                            po = psb.tile([P, OUT_DIM], F32, tag="pm", name="po")
                            for c in range(kt):
                                nc.tensor.matmul(
                                    out=po[:W2],
                                    lhsT=S[:, c * W2:(c + 1) * W2],
                                    rhs=G[:, cbase - gc0 + c, :],
                                    start=(c == 0), stop=(c == kt - 1),
                                    skip_group_check=True)
                            if h == 0:
                                nc.scalar.copy(
                                    out=acc[:W2, tt * OUT_DIM:(tt + 1) * OUT_DIM],
                                    in_=po[:W2])
                                cbase += kt
                                continue
                            rt0 = pw.tile([P, OUT_DIM], F32, tag="rt0", name="rt0")
                            nc.vector.tensor_tensor(
                                out=rt0[:nt], in0=po[:nt],
                                in1=acc[:nt, tt * OUT_DIM:(tt + 1) * OUT_DIM],
                                op=mybir.AluOpType.add)
                            rt = pw.tile([P, OUT_DIM], F32, tag="rt", name="rt")
                            if bias_nz:
                                nc.scalar.activation(
                                    out=rt[:nt], in_=rt0[:nt],
                                    func=mybir.ActivationFunctionType.Copy,
                                    scale=dinv_sb[:nt, tt:tt + 1])
                                nc.vector.tensor_tensor(
                                    out=rt[:nt], in0=rt[:nt], in1=bias_bc[:nt],
                                    op=mybir.AluOpType.add)
                                nc.vector.tensor_scalar(
                                    out=rt[:nt], in0=rt[:nt], scalar1=0.0,
                                    scalar2=None, op0=mybir.AluOpType.max)
                            else:
                                nc.scalar.activation(
                                    out=rt[:nt], in_=rt0[:nt],
                                    func=mybir.ActivationFunctionType.Relu,
                                    scale=dinv_sb[:nt, tt:tt + 1])
                            nc.tensor.matmul(
                                out=pcol[:OUT_DIM], lhsT=rt[:nt, :],
                                rhs=ones_f32[:nt, :],
                                start=first, stop=(tt == n_t2_d - 1),
                                skip_group_check=True)
                            first = False
                            cbase += kt
                ocol = pw.tile([P, 1], F32, tag="oc", name="oc")
                nc.vector.tensor_copy(out=ocol[:OUT_DIM], in_=pcol[:OUT_DIM])
                nc.sync.dma_start(out=out_part[:, :], in_=ocol[:OUT_DIM])
                ctx2.close()

        for _rep in range(reps):
            one_rep()

    nc.compile()
    return nc


def prepare_inputs(x, w, bias, hyperedge_index):
    npc, e_pad, n_t1, eslice, n_t2 = _derived()
    src = np.asarray(hyperedge_index[0], dtype=np.int64)
    edge = np.asarray(hyperedge_index[1], dtype=np.int64)

    deg_e = np.bincount(edge, minlength=e_pad).astype(np.float64)
    b_inv = np.where(deg_e > 0, 1.0 / np.maximum(deg_e, 1), 0.0).astype(np.float32)
    deg_n = np.bincount(src, minlength=N_NODES).astype(np.float64)
    d_inv = np.where(deg_n > 0, 1.0 / np.maximum(deg_n, 1), 0.0).astype(np.float32)

    half = e_pad // 2
    core_of = src // npc
    per_core = []
    cnt1 = np.zeros((NCORES, n_t1), np.int64)
    cnt2 = np.zeros((NCORES, 2, n_t2), np.int64)
    for c in range(NCORES):
        sel = core_of == c
        s_loc, e_glob = src[sel] - c * npc, edge[sel]
        per_core.append((s_loc, e_glob))
        cnt1[c] = np.bincount(e_glob // W1, minlength=n_t1)
        for h in range(2):
            hs = (e_glob >= half) == bool(h)
            cnt2[c, h] = np.bincount(s_loc[hs] // W2, minlength=n_t2)
    chunks1 = np.maximum(1, -(-cnt1.max(axis=0) // P))
    chunks2 = np.maximum(1, -(-cnt2.max(axis=(0,)) // P)).reshape(2 * n_t2)

    bias = np.asarray(bias, dtype=np.float32).reshape(1, -1)
    bias_nz = bool(np.any(bias != 0))
    x = np.asarray(x)
    w_bf = np.ascontiguousarray(np.asarray(w, dtype=np.float32).astype(
        ml_dtypes.bfloat16))

    in_maps = []
    for c, (s_loc, e_glob) in enumerate(per_core):
        g1, oh1, LA = _bucket_entries(
            s_loc, (e_glob % W1).astype(np.float32), e_glob // W1, n_t1, chunks1)
        g2p, oh2p = [], []
        for h in range(2):
            hs = (e_glob >= half) == bool(h)
            gh, ohh, _ = _bucket_entries(
                e_glob[hs] - h * half, (s_loc[hs] % W2).astype(np.float32),
                s_loc[hs] // W2, n_t2, chunks2[h * n_t2:(h + 1) * n_t2])
            g2p.append(gh)
            oh2p.append(ohh)
        g2 = np.concatenate(g2p)
        oh2 = np.concatenate(oh2p)

        xT = np.ascontiguousarray(
            x[c * npc:(c + 1) * npc].astype(np.float32).T.astype(ml_dtypes.bfloat16))
        esl_h = e_pad // 2 // NCORES
        cols = []
        for h in range(2):
            base = h * (e_pad // 2) + c * esl_h
            cols.append(b_inv[base:base + esl_h].reshape(-1, P).T)
        binv_c = np.ascontiguousarray(np.concatenate(cols, axis=1))
        dinv_c = np.zeros((P, n_t2), np.float32)
        dloc = np.zeros(n_t2 * W2, np.float32)
        dloc[:npc] = d_inv[c * npc:(c + 1) * npc]
        dinv_c[:W2, :] = dloc.reshape(n_t2, W2).T
        in_maps.append({
            "xT": xT,
            "w": w_bf,
            "bias": np.ascontiguousarray(bias),
            "idxA": _wrap_idx16(g1),
            "ohA": _oh_cols(oh1),
            "idxB": _wrap_idx16(g2),
            "ohB": _oh_cols(oh2),
            "binv": binv_c,
            "dinv": np.ascontiguousarray(dinv_c),
        })

    return in_maps, chunks1, chunks2, bias_nz


def kernel(x_node_features, lin_weight, bias, hyperedge_index):
    in_maps, chunks1, chunks2, bias_nz = prepare_inputs(
        x_node_features, lin_weight, bias, hyperedge_index)
    nc = build_kernel(chunks1, chunks2, bias_nz)
    res = run_bass_kernel_spmd(nc, in_maps, list(range(NCORES)))
    total = np.zeros(OUT_DIM, np.float64)
    for c in range(NCORES):
        total += res.results[c]["out_part"][:, 0].astype(np.float64)
    return (total / N_NODES).astype(np.float32)
